# revision 15
# baseline (speedup 1.0000x reference)
"""CapsNet forward on 8 trn2 NeuronCores — data-parallel convs on device.

Per-core device kernel (SPMD over batch): build conv1 im2col on device
from raw images, conv1 as one 82x256 GEMM (+bias row) + relu, primary
caps conv as 162 accumulated matmuls with the 1/8-sharded weights
AllGathered across cores on device. Host: squash + dynamic routing,
reformulated as small GEMMs that never materialize u_hat.

All inputs ship as a single bf16 blob per core (one sharded transfer);
the donated output buffer is created on device so it never crosses the
axon link. The jitted executable is built and warmed at import time in
a background thread.
"""
import threading
import numpy as np
import ml_dtypes

B = 256
NCORES = 8
BL = B // NCORES          # 32 images per core
POS1 = 32 * 20 * 20       # conv1 output positions per core (img,oh,ow)
KHW = 81
K1 = 82                   # 81 taps + bias row
NPOS2 = 36                # 6x6
CHUNKS = [(0, 12), (12, 12), (24, 8)]
W2COLS = KHW * 256        # 20736

IMG_OFF = 0
IMG_N = BL * 784          # 25088
W1T_OFF = IMG_N
W1T_N = K1 * 256          # 20992
W2P_OFF = W1T_OFF + W1T_N
W2P_N = 32 * W2COLS       # 663552
BLOB_N = W2P_OFF + W2P_N  # 709632

_exec_time_ns = None
_rt = {}                  # runtime state: nc, sharded jit, premade zeros...
_warm_lock = threading.Lock()


def _build():
    """Build + bass-compile the per-core kernel. No device access needed."""
    import concourse.bass as bass
    import concourse.bacc as bacc
    import concourse.mybir as mybir
    import concourse.tile as tile

    bf16 = mybir.dt.bfloat16
    f32 = mybir.dt.float32
    AF = mybir.ActivationFunctionType

    nc = bacc.Bacc("TRN2", target_bir_lowering=False, debug=False,
                   enable_asserts=False, num_devices=NCORES)
    blob_d = nc.dram_tensor("blob", [BLOB_N], bf16, kind="ExternalInput")
    uout_d = nc.dram_tensor("uout", [2, 128, BL * NPOS2], bf16,
                            kind="ExternalOutput")

    with tile.TileContext(nc) as tc:
        with tc.tile_pool(name="const", bufs=1) as const, \
             tc.tile_pool(name="dram", bufs=1, space="DRAM") as dramp, \
             tc.tile_pool(name="ps1", bufs=2, space="PSUM") as ps1, \
             tc.tile_pool(name="ps2", bufs=3, space="PSUM") as ps2, \
             tc.tile_pool(name="outp", bufs=3) as outp:
            # primary-caps weights: each core holds rows [32c, 32c+32) of the
            # [256, 20736] (in_ch, tap*256+out_ch) matrix; AllGather the rest.
            w2pb = dramp.tile([32, W2COLS], bf16, name="w2pb")
            w2full = dramp.tile([256, W2COLS], bf16, addr_space="Shared",
                                name="w2full")
            nc.gpsimd.dma_start(
                w2pb[:].flatten(), bass.AP(blob_d, W2P_OFF, [[1, W2P_N]]))
            nc.gpsimd.collective_compute(
                "AllGather", mybir.AluOpType.bypass,
                replica_groups=[list(range(NCORES))],
                ins=[w2pb.opt()], outs=[w2full.opt()])
            w2_sb = []
            for ci in range(2):
                t = const.tile([128, W2COLS], bf16, name=f"w2_{ci}")
                nc.sync.dma_start(t[:], w2full[ci * 128:(ci + 1) * 128, :])
                w2_sb.append(t)

            # conv1 im2col on device: row kh*9+kw of [81, (img,oh,ow)] is the
            # overlapping 20x20 window of each image at tap offset (kh,kw);
            # row 81 is all-ones for the bias.
            im2col_sb = const.tile([K1, POS1], bf16, name="im2col")
            for kh in range(9):
                for kw in range(9):
                    src = bass.AP(blob_d, IMG_OFF + kh * 28 + kw,
                                  [[784, BL], [28, 20], [1, 20]])
                    r = kh * 9 + kw
                    nc.sync.dma_start(im2col_sb[r:r + 1, :], src)
            nc.vector.memset(im2col_sb[KHW:K1, :], 1.0)
            w1t_sb = const.tile([K1, 256], bf16, name="w1t")
            nc.sync.dma_start(w1t_sb[:],
                              bass.AP(blob_d, W1T_OFF, [[256, K1], [1, 256]]))

            # conv1 + bias + relu: out[oc, (img,oh,ow)] = relu(W1.T @ im2col)
            x1 = [const.tile([128, POS1], bf16, name=f"x1_{ot}")
                  for ot in range(2)]
            for ot in range(2):
                for c in range(POS1 // 512):
                    ps = ps1.tile([128, 512], f32, name="c1", tag="c1")
                    nc.tensor.matmul(
                        ps[:], w1t_sb[:, ot * 128:(ot + 1) * 128],
                        im2col_sb[:, c * 512:(c + 1) * 512],
                        start=True, stop=True)
                    nc.scalar.activation(
                        x1[ot][:, c * 512:(c + 1) * 512], ps[:], AF.Relu)

            # primary caps conv: stride 2, 9x9, 256->256, accumulate 162 matmuls
            x1v = [x1[ot][:].rearrange("p (b h w) -> p b h w", b=BL, h=20, w=20)
                   for ot in range(2)]
            for ot in range(2):
                pss = [ps2.tile([128, nb * NPOS2], f32, name=f"c2_{ot}_{ic}",
                                tag="c2")
                       for ic, (b0, nb) in enumerate(CHUNKS)]
                nk = 0
                for kh in range(9):
                    for kw in range(9):
                        for ci in range(2):
                            khkw = kh * 9 + kw
                            lhsT = w2_sb[ci][:, khkw * 256 + ot * 128:
                                             khkw * 256 + ot * 128 + 128]
                            for ic, (b0, nb) in enumerate(CHUNKS):
                                rhs = x1v[ci][:, b0:b0 + nb,
                                              kh:kh + 11:2, kw:kw + 11:2]
                                nc.tensor.matmul(pss[ic][:], lhsT, rhs,
                                                 start=(nk == 0), stop=(nk == 161))
                            nk += 1
                for ic, (b0, nb) in enumerate(CHUNKS):
                    ob = outp.tile([128, nb * NPOS2], bf16,
                                   name=f"ob_{ot}_{ic}", tag="ob")
                    nc.scalar.activation(ob[:], pss[ic][:], AF.Copy)
                    nc.sync.dma_start(
                        uout_d.ap()[ot][:, b0 * NPOS2:(b0 + nb) * NPOS2], ob[:])

    nc.compile()
    return nc


def _make_runner(nc):
    """Persistent jitted SPMD executable (the same path run_bass_kernel_spmd
    takes under axon, with the jit + donated output buffer kept alive)."""
    import jax
    import jax.numpy as jnp
    from jax.sharding import Mesh, PartitionSpec, NamedSharding
    from jax.experimental.shard_map import shard_map
    import concourse.mybir as mybir
    from concourse import bass2jax

    bass2jax.install_neuronx_cc_hook()

    in_names, out_names, out_avals = [], [], []
    partition_name = (nc.partition_id_tensor.name
                      if nc.partition_id_tensor else None)
    for alloc in nc.m.functions[0].allocations:
        if not isinstance(alloc, mybir.MemoryLocationSet):
            continue
        name = alloc.memorylocations[0].name
        if alloc.kind == "ExternalInput":
            if name != partition_name:
                in_names.append(name)
        elif alloc.kind == "ExternalOutput":
            out_names.append(name)
            out_avals.append(jax.core.ShapedArray(
                tuple(alloc.tensor_shape), mybir.dt.np(alloc.dtype)))
    assert in_names == ["blob"] and out_names == ["uout"], (in_names, out_names)
    all_in_names = in_names + out_names
    if partition_name is not None:
        all_in_names.append(partition_name)

    def _body(*args):
        operands = list(args)
        if partition_name is not None:
            operands.append(bass2jax.partition_id_tensor())
        outs = bass2jax._bass_exec_p.bind(
            *operands,
            out_avals=tuple(out_avals),
            in_names=tuple(all_in_names),
            out_names=tuple(out_names),
            lowering_input_output_aliases=(),
            sim_require_finite=True,
            sim_require_nnan=True,
            nc=nc,
        )
        return tuple(outs)

    devices = jax.devices()[:NCORES]
    mesh = Mesh(np.asarray(devices), ("core",))
    sharded = jax.jit(
        shard_map(_body, mesh=mesh,
                  in_specs=(PartitionSpec("core"),) * 2,
                  out_specs=(PartitionSpec("core"),),
                  check_rep=False),
        donate_argnums=(1,), keep_unused=True)
    zshape, zdtype = ((NCORES * 2, 128, BL * NPOS2),
                      mybir.dt.np(nc.lookup_mls("uout").dtype))
    make_zeros = jax.jit(
        lambda: jnp.zeros(zshape, zdtype),
        out_shardings=NamedSharding(mesh, PartitionSpec("core")))
    return sharded, make_zeros


def _warmup():
    """Build, compile, jit, and run once with dummy data so the NEFF cache,
    jit cache, and a donated output buffer are all hot before kernel()."""
    with _warm_lock:
        if "err" in _rt:
            del _rt["err"]
        try:
            if "nc" not in _rt:
                _rt["nc"] = _build()
            if "sharded" not in _rt:
                _rt["sharded"], _rt["make_zeros"] = _make_runner(_rt["nc"])
            import jax
            if not _rt.get("warm"):
                dummy = np.zeros(NCORES * BLOB_N, ml_dtypes.bfloat16)
                out = _rt["sharded"](dummy, _rt["make_zeros"]())
                jax.block_until_ready(out)
                _rt["warm"] = True
            if "zeros" not in _rt:
                z = _rt["make_zeros"]()
                jax.block_until_ready(z)
                _rt["zeros"] = z
        except Exception as e:
            import traceback
            traceback.print_exc()
            _rt["err"] = e


def _stage_blob(images, conv1_w, conv1_b, prim_w):
    bf = ml_dtypes.bfloat16
    blob = np.empty((NCORES, BLOB_N), bf)
    blob[:, :W1T_OFF] = images.reshape(NCORES, IMG_N).astype(bf)
    w1tb = np.empty((K1, 256), np.float32)
    w1tb[:KHW] = conv1_w.reshape(256, KHW).T
    w1tb[KHW] = conv1_b
    blob[:, W1T_OFF:W2P_OFF] = w1tb.reshape(-1).astype(bf)
    w2full = np.ascontiguousarray(
        prim_w.reshape(256, 256, KHW).transpose(1, 2, 0)).reshape(NCORES, W2P_N)
    blob[:, W2P_OFF:] = w2full.astype(bf)
    return blob


def _run_device(blob):
    """blob: [NCORES, BLOB_N] bf16 -> list of per-core uout [2,128,1152] f32."""
    global _exec_time_ns
    import jax
    if not _rt.get("warm") or "err" in _rt:
        _warmup()
    if "err" in _rt:
        raise _rt["err"]
    z = _rt.pop("zeros", None)
    if z is None:
        z = _rt["make_zeros"]()
    outs = _rt["sharded"](blob.reshape(-1), z)
    jax.block_until_ready(outs)
    uout = outs[0]
    import concurrent.futures as cf
    with cf.ThreadPoolExecutor(NCORES) as ex:
        shards = sorted(uout.addressable_shards,
                        key=lambda s: s.index[0].start or 0)
        res = list(ex.map(lambda s: np.asarray(s.data, np.float32), shards))
    return res


def _run_device_spmd_fallback(blob):
    """Fallback: the stock run_bass_kernel_spmd path."""
    global _exec_time_ns
    from concourse.bass_utils import run_bass_kernel_spmd
    with _warm_lock:
        if "nc" not in _rt:
            _rt["nc"] = _build()
    in_maps = [{"blob": blob[c]} for c in range(NCORES)]
    res = run_bass_kernel_spmd(_rt["nc"], in_maps, core_ids=list(range(NCORES)))
    _exec_time_ns = res.exec_time_ns
    return [res.results[c]["uout"].astype(np.float32) for c in range(NCORES)]


def _host_conv_fallback(images, conv1_w, conv1_b, prim_w):
    outs = []
    w1 = conv1_w.reshape(256, KHW)
    wfull = np.ascontiguousarray(
        prim_w.reshape(256, 256 * KHW).T)            # [(ic,tap), oc]
    for c in range(NCORES):
        img = images[c * BL:(c + 1) * BL, 0]
        sw = np.lib.stride_tricks.sliding_window_view(img, (9, 9), axis=(1, 2))
        a = sw.transpose(3, 4, 0, 1, 2).reshape(KHW, POS1)
        x1 = np.maximum(w1 @ a + conv1_b[:, None], 0.0).reshape(256, BL, 20, 20)
        patches = np.empty((256 * KHW, BL * NPOS2), np.float32)
        for kh in range(9):
            for kw in range(9):
                khkw = kh * 9 + kw
                sl = x1[:, :, kh:kh + 11:2, kw:kw + 11:2].reshape(256, -1)
                patches.reshape(256, KHW, -1)[:, khkw] = sl
        acc = wfull.T @ patches.reshape(256 * KHW, -1)
        outs.append(acc.reshape(2, 128, BL * NPOS2))
    return outs


def _routing(u, W):
    """Dynamic routing without materializing u_hat.

    u: [B, 1152, 8] squashed primary caps; W: [1152, 10, 16, 8].
    s_j and the agreement both factor through GEMMs on u2 = u.[B, 9216]:
      s[b,(j,d)] = sum_{r,i} u2[b,(r,i)] * c[r,j] * W[r,j,d,i]
      agree[r,j] = sum_{i,d} W[r,j,d,i] * G[(r,i),(j,d)],  G = u2.T @ v / B
    """
    u2 = np.ascontiguousarray(u.reshape(B, 1152 * 8))
    Wt = np.ascontiguousarray(W.transpose(1, 2, 0, 3)).reshape(160, 9216)
    Wr = np.ascontiguousarray(W.transpose(0, 3, 1, 2)).reshape(1152, 8, 160)
    b_ij = np.zeros((1152, 10), np.float32)
    v = None
    for it in range(3):
        e = np.exp(b_ij - b_ij.max(axis=1, keepdims=True))
        c = e / e.sum(axis=1, keepdims=True)
        M = (Wt.reshape(10, 16, 1152, 8) * c.T[:, None, :, None]) \
            .reshape(160, 9216)
        s = (u2 @ M.T).reshape(B, 10, 16)
        sq = np.sum(s * s, axis=2, keepdims=True)
        v = sq / (1.0 + sq) * (s / np.sqrt(sq))
        if it == 2:
            break
        G = (u2.T @ v.reshape(B, 160)) * (1.0 / B)   # [9216, 160]
        agree = (Wr * G.reshape(1152, 8, 160)).sum(axis=1) \
            .reshape(1152, 10, 16).sum(axis=2)
        b_ij = b_ij + agree
    return v


def kernel(images, labels, conv1_w, conv1_b, prim_w, prim_b, W):
    images = np.asarray(images, np.float32)
    conv1_w = np.asarray(conv1_w, np.float32)
    conv1_b = np.asarray(conv1_b, np.float32)
    prim_w = np.asarray(prim_w, np.float32)
    prim_b = np.asarray(prim_b, np.float32)
    W = np.asarray(W, np.float32)

    blob = _stage_blob(images, conv1_w, conv1_b, prim_w)
    t = _rt.get("thread")
    if t is not None and t.is_alive():
        t.join()
    try:
        uouts = _run_device(blob)
    except Exception as e:
        import traceback
        traceback.print_exc()
        print("CACHED-JIT PATH FAILED — trying run_bass_kernel_spmd:", e)
        try:
            uouts = _run_device_spmd_fallback(blob)
        except Exception as e2:
            traceback.print_exc()
            print("DEVICE PATH FAILED — numpy fallback:", e2)
            uouts = _host_conv_fallback(images, conv1_w, conv1_b, prim_w)

    # u[b, g, m*36+pos] from uout[ot, oc, b*36+pos], ch = ot*128+oc = g*32+m
    us = []
    for c in range(NCORES):
        y = uouts[c].reshape(256, BL, NPOS2) + prim_b[:, None, None]
        u = y.reshape(8, 32, BL, NPOS2).transpose(2, 0, 1, 3).reshape(BL, 8, 1152)
        us.append(u)
    u = np.concatenate(us, 0).transpose(0, 2, 1)               # [B,1152,8]

    sq = np.sum(u * u, axis=1, keepdims=True)                  # [B,1,8]
    u = sq / (1.0 + sq) * (u / np.sqrt(sq))
    v = _routing(np.ascontiguousarray(u), W)
    return v[..., None].astype(np.float32)


def _start_warmup():
    t = threading.Thread(target=_warmup, daemon=True)
    t.start()
    _rt["thread"] = t


_start_warmup()


# revision 17
# speedup vs baseline: 4.8899x; 4.8899x over previous
"""CapsNet forward on 8 trn2 NeuronCores — data-parallel convs on device.

Per-core device kernel (SPMD over batch): build conv1 im2col on device
from raw images, conv1 as one 82x256 GEMM (+bias row) + relu, primary
caps conv as 162 accumulated matmuls with the 1/8-sharded weights
AllGathered across cores on device. Host: squash + dynamic routing,
reformulated as small GEMMs that never materialize u_hat.

All inputs ship as a single bf16 blob per core (one sharded transfer);
the donated output buffer is created on device so it never crosses the
axon link. The jitted executable is built and warmed at import time in
a background thread.
"""
import threading
import numpy as np
import ml_dtypes

B = 256
NCORES = 8
BL = B // NCORES          # 32 images per core
POS1 = 32 * 20 * 20       # conv1 output positions per core (img,oh,ow)
KHW = 81
K1 = 82                   # 81 taps + bias row
NPOS2 = 36                # 6x6
CHUNKS = [(0, 12), (12, 12), (24, 8)]
W2COLS = KHW * 256        # 20736

IMG_OFF = 0
IMG_N = BL * 784          # 25088
W1T_OFF = IMG_N
W1T_N = K1 * 256          # 20992
W2P_OFF = W1T_OFF + W1T_N
W2P_N = 32 * W2COLS       # 663552
BLOB_N = W2P_OFF + W2P_N  # 709632

_exec_time_ns = None
_rt = {}                  # runtime state: nc, sharded jit, premade zeros...
_warm_lock = threading.Lock()


def _build():
    """Build + bass-compile the per-core kernel. No device access needed."""
    import concourse.bass as bass
    import concourse.bacc as bacc
    import concourse.mybir as mybir
    import concourse.tile as tile

    bf16 = mybir.dt.bfloat16
    f32 = mybir.dt.float32
    AF = mybir.ActivationFunctionType

    nc = bacc.Bacc("TRN2", target_bir_lowering=False, debug=False,
                   enable_asserts=False, num_devices=NCORES)
    blob_d = nc.dram_tensor("blob", [BLOB_N], bf16, kind="ExternalInput")
    uout_d = nc.dram_tensor("uout", [2, 128, BL * NPOS2], bf16,
                            kind="ExternalOutput")

    with tile.TileContext(nc) as tc:
        with tc.tile_pool(name="const", bufs=1) as const, \
             tc.tile_pool(name="dram", bufs=1, space="DRAM") as dramp, \
             tc.tile_pool(name="ps1", bufs=2, space="PSUM") as ps1, \
             tc.tile_pool(name="ps2", bufs=3, space="PSUM") as ps2, \
             tc.tile_pool(name="outp", bufs=3) as outp:
            # primary-caps weights: each core holds rows [32c, 32c+32) of the
            # [256, 20736] (in_ch, tap*256+out_ch) matrix; AllGather the rest.
            w2pb = dramp.tile([32, W2COLS], bf16, name="w2pb")
            w2full = dramp.tile([256, W2COLS], bf16, addr_space="Shared",
                                name="w2full")
            nc.gpsimd.dma_start(
                w2pb[:].flatten(), bass.AP(blob_d, W2P_OFF, [[1, W2P_N]]))
            nc.gpsimd.collective_compute(
                "AllGather", mybir.AluOpType.bypass,
                replica_groups=[list(range(NCORES))],
                ins=[w2pb.opt()], outs=[w2full.opt()])
            w2_sb = []
            for ci in range(2):
                t = const.tile([128, W2COLS], bf16, name=f"w2_{ci}")
                nc.sync.dma_start(t[:], w2full[ci * 128:(ci + 1) * 128, :])
                w2_sb.append(t)

            # conv1 im2col on device: row 1 + kh*9+kw of [82, (img,oh,ow)] is
            # the overlapping 20x20 window of each image at tap offset
            # (kh,kw); row 0 is all-ones for the bias (partition 0 so the
            # memset's partition base is quadrant-aligned).
            im2col_sb = const.tile([K1, POS1], bf16, name="im2col")
            for kh in range(9):
                for kw in range(9):
                    src = bass.AP(blob_d, IMG_OFF + kh * 28 + kw,
                                  [[784, BL], [28, 20], [1, 20]])
                    r = 1 + kh * 9 + kw
                    nc.sync.dma_start(im2col_sb[r:r + 1, :], src)
            nc.vector.memset(im2col_sb[0:1, :], 1.0)
            w1t_sb = const.tile([K1, 256], bf16, name="w1t")
            nc.sync.dma_start(w1t_sb[:],
                              bass.AP(blob_d, W1T_OFF, [[256, K1], [1, 256]]))

            # conv1 + bias + relu: out[oc, (img,oh,ow)] = relu(W1.T @ im2col)
            x1 = [const.tile([128, POS1], bf16, name=f"x1_{ot}")
                  for ot in range(2)]
            for ot in range(2):
                for c in range(POS1 // 512):
                    ps = ps1.tile([128, 512], f32, name="c1", tag="c1")
                    nc.tensor.matmul(
                        ps[:], w1t_sb[:, ot * 128:(ot + 1) * 128],
                        im2col_sb[:, c * 512:(c + 1) * 512],
                        start=True, stop=True)
                    nc.scalar.activation(
                        x1[ot][:, c * 512:(c + 1) * 512], ps[:], AF.Relu)

            # primary caps conv: stride 2, 9x9, 256->256, accumulate 162 matmuls
            x1v = [x1[ot][:].rearrange("p (b h w) -> p b h w", b=BL, h=20, w=20)
                   for ot in range(2)]
            for ot in range(2):
                pss = [ps2.tile([128, nb * NPOS2], f32, name=f"c2_{ot}_{ic}",
                                tag="c2")
                       for ic, (b0, nb) in enumerate(CHUNKS)]
                nk = 0
                for kh in range(9):
                    for kw in range(9):
                        for ci in range(2):
                            khkw = kh * 9 + kw
                            lhsT = w2_sb[ci][:, khkw * 256 + ot * 128:
                                             khkw * 256 + ot * 128 + 128]
                            for ic, (b0, nb) in enumerate(CHUNKS):
                                rhs = x1v[ci][:, b0:b0 + nb,
                                              kh:kh + 11:2, kw:kw + 11:2]
                                nc.tensor.matmul(pss[ic][:], lhsT, rhs,
                                                 start=(nk == 0), stop=(nk == 161))
                            nk += 1
                for ic, (b0, nb) in enumerate(CHUNKS):
                    ob = outp.tile([128, nb * NPOS2], bf16,
                                   name=f"ob_{ot}_{ic}", tag="ob")
                    nc.scalar.activation(ob[:], pss[ic][:], AF.Copy)
                    nc.sync.dma_start(
                        uout_d.ap()[ot][:, b0 * NPOS2:(b0 + nb) * NPOS2], ob[:])

    nc.compile()
    return nc


def _make_runner(nc):
    """Persistent jitted SPMD executable (the same path run_bass_kernel_spmd
    takes under axon, with the jit + donated output buffer kept alive)."""
    import jax
    import jax.numpy as jnp
    from jax.sharding import Mesh, PartitionSpec, NamedSharding
    from jax.experimental.shard_map import shard_map
    import concourse.mybir as mybir
    from concourse import bass2jax

    bass2jax.install_neuronx_cc_hook()

    in_names, out_names, out_avals = [], [], []
    partition_name = (nc.partition_id_tensor.name
                      if nc.partition_id_tensor else None)
    for alloc in nc.m.functions[0].allocations:
        if not isinstance(alloc, mybir.MemoryLocationSet):
            continue
        name = alloc.memorylocations[0].name
        if alloc.kind == "ExternalInput":
            if name != partition_name:
                in_names.append(name)
        elif alloc.kind == "ExternalOutput":
            out_names.append(name)
            out_avals.append(jax.core.ShapedArray(
                tuple(alloc.tensor_shape), mybir.dt.np(alloc.dtype)))
    assert in_names == ["blob"] and out_names == ["uout"], (in_names, out_names)
    all_in_names = in_names + out_names
    if partition_name is not None:
        all_in_names.append(partition_name)

    def _body(*args):
        operands = list(args)
        if partition_name is not None:
            operands.append(bass2jax.partition_id_tensor())
        outs = bass2jax._bass_exec_p.bind(
            *operands,
            out_avals=tuple(out_avals),
            in_names=tuple(all_in_names),
            out_names=tuple(out_names),
            lowering_input_output_aliases=(),
            sim_require_finite=True,
            sim_require_nnan=True,
            nc=nc,
        )
        return tuple(outs)

    devices = jax.devices()[:NCORES]
    mesh = Mesh(np.asarray(devices), ("core",))
    sharded = jax.jit(
        shard_map(_body, mesh=mesh,
                  in_specs=(PartitionSpec("core"),) * 2,
                  out_specs=(PartitionSpec("core"),),
                  check_rep=False),
        donate_argnums=(1,), keep_unused=True)
    zshape, zdtype = ((NCORES * 2, 128, BL * NPOS2),
                      mybir.dt.np(nc.lookup_mls("uout").dtype))
    make_zeros = jax.jit(
        lambda: jnp.zeros(zshape, zdtype),
        out_shardings=NamedSharding(mesh, PartitionSpec("core")))
    return sharded, make_zeros


def _warmup():
    """Build, compile, jit, and run once with dummy data so the NEFF cache,
    jit cache, and a donated output buffer are all hot before kernel()."""
    with _warm_lock:
        if "err" in _rt:
            del _rt["err"]
        try:
            if "nc" not in _rt:
                _rt["nc"] = _build()
            if "sharded" not in _rt:
                _rt["sharded"], _rt["make_zeros"] = _make_runner(_rt["nc"])
            import jax
            if not _rt.get("warm"):
                dummy = np.zeros(NCORES * BLOB_N, ml_dtypes.bfloat16)
                out = _rt["sharded"](dummy, _rt["make_zeros"]())
                jax.block_until_ready(out)
                _rt["warm"] = True
            if "zeros" not in _rt:
                z = _rt["make_zeros"]()
                jax.block_until_ready(z)
                _rt["zeros"] = z
        except Exception as e:
            import traceback
            traceback.print_exc()
            _rt["err"] = e


def _stage_blob(images, conv1_w, conv1_b, prim_w):
    bf = ml_dtypes.bfloat16
    blob = np.empty((NCORES, BLOB_N), bf)
    blob[:, :W1T_OFF] = images.reshape(NCORES, IMG_N).astype(bf)
    w1tb = np.empty((K1, 256), np.float32)
    w1tb[0] = conv1_b
    w1tb[1:] = conv1_w.reshape(256, KHW).T
    blob[:, W1T_OFF:W2P_OFF] = w1tb.reshape(-1).astype(bf)
    w2full = np.ascontiguousarray(
        prim_w.reshape(256, 256, KHW).transpose(1, 2, 0)).reshape(NCORES, W2P_N)
    blob[:, W2P_OFF:] = w2full.astype(bf)
    return blob


def _run_device(blob):
    """blob: [NCORES, BLOB_N] bf16 -> list of per-core uout [2,128,1152] f32."""
    global _exec_time_ns
    import jax
    if not _rt.get("warm") or "err" in _rt:
        _warmup()
    if "err" in _rt:
        raise _rt["err"]
    z = _rt.pop("zeros", None)
    if z is None:
        z = _rt["make_zeros"]()
    outs = _rt["sharded"](blob.reshape(-1), z)
    jax.block_until_ready(outs)
    uout = outs[0]
    import concurrent.futures as cf
    with cf.ThreadPoolExecutor(NCORES) as ex:
        shards = sorted(uout.addressable_shards,
                        key=lambda s: s.index[0].start or 0)
        res = list(ex.map(lambda s: np.asarray(s.data, np.float32), shards))
    return res


def _run_device_spmd_fallback(blob):
    """Fallback: the stock run_bass_kernel_spmd path."""
    global _exec_time_ns
    from concourse.bass_utils import run_bass_kernel_spmd
    with _warm_lock:
        if "nc" not in _rt:
            _rt["nc"] = _build()
    in_maps = [{"blob": blob[c]} for c in range(NCORES)]
    res = run_bass_kernel_spmd(_rt["nc"], in_maps, core_ids=list(range(NCORES)))
    _exec_time_ns = res.exec_time_ns
    return [res.results[c]["uout"].astype(np.float32) for c in range(NCORES)]


def _host_conv_fallback(images, conv1_w, conv1_b, prim_w):
    outs = []
    w1 = conv1_w.reshape(256, KHW)
    wfull = np.ascontiguousarray(
        prim_w.reshape(256, 256 * KHW).T)            # [(ic,tap), oc]
    for c in range(NCORES):
        img = images[c * BL:(c + 1) * BL, 0]
        sw = np.lib.stride_tricks.sliding_window_view(img, (9, 9), axis=(1, 2))
        a = sw.transpose(3, 4, 0, 1, 2).reshape(KHW, POS1)
        x1 = np.maximum(w1 @ a + conv1_b[:, None], 0.0).reshape(256, BL, 20, 20)
        patches = np.empty((256 * KHW, BL * NPOS2), np.float32)
        for kh in range(9):
            for kw in range(9):
                khkw = kh * 9 + kw
                sl = x1[:, :, kh:kh + 11:2, kw:kw + 11:2].reshape(256, -1)
                patches.reshape(256, KHW, -1)[:, khkw] = sl
        acc = wfull.T @ patches.reshape(256 * KHW, -1)
        outs.append(acc.reshape(2, 128, BL * NPOS2))
    return outs


def _routing(u, W):
    """Dynamic routing without materializing u_hat.

    u: [B, 1152, 8] squashed primary caps; W: [1152, 10, 16, 8].
    s_j and the agreement both factor through GEMMs on u2 = u.[B, 9216]:
      s[b,(j,d)] = sum_{r,i} u2[b,(r,i)] * c[r,j] * W[r,j,d,i]
      agree[r,j] = sum_{i,d} W[r,j,d,i] * G[(r,i),(j,d)],  G = u2.T @ v / B
    """
    u2 = np.ascontiguousarray(u.reshape(B, 1152 * 8))
    Wt = np.ascontiguousarray(W.transpose(1, 2, 0, 3)).reshape(160, 9216)
    Wr = np.ascontiguousarray(W.transpose(0, 3, 1, 2)).reshape(1152, 8, 160)
    b_ij = np.zeros((1152, 10), np.float32)
    v = None
    for it in range(3):
        e = np.exp(b_ij - b_ij.max(axis=1, keepdims=True))
        c = e / e.sum(axis=1, keepdims=True)
        M = (Wt.reshape(10, 16, 1152, 8) * c.T[:, None, :, None]) \
            .reshape(160, 9216)
        s = (u2 @ M.T).reshape(B, 10, 16)
        sq = np.sum(s * s, axis=2, keepdims=True)
        v = sq / (1.0 + sq) * (s / np.sqrt(sq))
        if it == 2:
            break
        G = (u2.T @ v.reshape(B, 160)) * (1.0 / B)   # [9216, 160]
        agree = (Wr * G.reshape(1152, 8, 160)).sum(axis=1) \
            .reshape(1152, 10, 16).sum(axis=2)
        b_ij = b_ij + agree
    return v


def kernel(images, labels, conv1_w, conv1_b, prim_w, prim_b, W):
    images = np.asarray(images, np.float32)
    conv1_w = np.asarray(conv1_w, np.float32)
    conv1_b = np.asarray(conv1_b, np.float32)
    prim_w = np.asarray(prim_w, np.float32)
    prim_b = np.asarray(prim_b, np.float32)
    W = np.asarray(W, np.float32)

    blob = _stage_blob(images, conv1_w, conv1_b, prim_w)
    t = _rt.get("thread")
    if t is not None and t.is_alive():
        t.join()
    try:
        uouts = _run_device(blob)
    except Exception as e:
        import traceback
        traceback.print_exc()
        print("CACHED-JIT PATH FAILED — trying run_bass_kernel_spmd:", e)
        try:
            uouts = _run_device_spmd_fallback(blob)
        except Exception as e2:
            traceback.print_exc()
            print("DEVICE PATH FAILED — numpy fallback:", e2)
            uouts = _host_conv_fallback(images, conv1_w, conv1_b, prim_w)

    # u[b, g, m*36+pos] from uout[ot, oc, b*36+pos], ch = ot*128+oc = g*32+m
    us = []
    for c in range(NCORES):
        y = uouts[c].reshape(256, BL, NPOS2) + prim_b[:, None, None]
        u = y.reshape(8, 32, BL, NPOS2).transpose(2, 0, 1, 3).reshape(BL, 8, 1152)
        us.append(u)
    u = np.concatenate(us, 0).transpose(0, 2, 1)               # [B,1152,8]

    sq = np.sum(u * u, axis=1, keepdims=True)                  # [B,1,8]
    u = sq / (1.0 + sq) * (u / np.sqrt(sq))
    v = _routing(np.ascontiguousarray(u), W)
    return v[..., None].astype(np.float32)


def _start_warmup():
    t = threading.Thread(target=_warmup, daemon=True)
    t.start()
    _rt["thread"] = t


_start_warmup()
